# revision 35
# baseline (speedup 1.0000x reference)
"""Multi-head attention (B=4, N=2048, DIM=512, H=8) on 8 TRN2 NeuronCores.

Sharding: core c handles (batch = c//2, query-half = c%2) -> 1024 queries of
one batch, all heads. Zero collectives: K/V are recomputed per core pair
(keys are permuted so each core's queries come first; softmax is
permutation-invariant over keys).

Device layout ("transposed flash"):
  - everything dim-major: X^T, Q^T, K^T in SBUF with the contraction dim on
    partitions; V in natural [n, d] layout with a fused ones-column so the
    PV matmul also produces the softmax denominators (row 64 of the PSUM
    accumulator).
  - scores S^T = K_h^T-stationary @ Q_h^T-moving -> [nk_tile, nq] PSUM.
    The two heads of a pair live on disjoint PE row groups (d on partitions
    0:64 vs 64:128); their score matmuls are emitted ADJACENTLY
    (A-c0, B-c0, A-c1, B-c1) so the PE runs them concurrently as 64x128
    row tiles (2x throughput -- verified on HW: 111 ns vs 217 ns per MM).
  - exp on ScalarE (scale fused) -> P^T bf16 in SBUF. A tunable subset of
    head-B tiles runs instead on VectorE via a custom 8-slice DVE op
    computing exp(x) ~= ((1 + u + u^2/2)^8, u = x/8 (rel err < 2e-3 over
    the logit range; softmax denominator cancels most of it), offloading
    the ScalarE bottleneck.
  - O^T_aug += V_aug^T-stationary @ P^T-moving accumulated over nk tiles.
  - normalization: reciprocal of the sums row, broadcast across partitions
    via a DRAM round-trip DMA (step-0 access pattern), multiply on VectorE.
  - output projection Wout-stationary gives Y^T [512, 1024]; host transposes.
"""

import os

import numpy as np
import ml_dtypes

B, N, DIM = 4, 2048, 512
H, D = 8, 64
NQ = 1024            # queries per core
NCORES = 8
SCALE = DIM ** -0.5  # reference scales by full dim, not head dim

BF16 = ml_dtypes.bfloat16

_CACHE = {}

LAST_EXEC_TIME_NS = None

# head-B's exp runs on the DVE for ALL k-tiles: symmetric engine use makes
# both ss PSUM slots free at the same time, so the next tile's A/B score
# matmuls become ready together and the PE pairs them (64x128 row tiles)
EXP8_C1 = 0.51        # tuned quadratic coefficient


def _register_exp8():
    """Register the EXP8_APPROX_ANT custom DVE op (idempotent).

    out = ((C1*u + 1) * u + 1)^8 with u = Src0 * C0; C0 folds the softmax
    scale / 8.  Exactly 8 ALU slices -- fits the v3 (TRN2) DVE pipeline.
    """
    import concourse.dve_ops as dve_ops
    from concourse.dve_spec import C0, C1, C2, Spec, lower, sq
    from concourse.dve_ops import DveOp
    from concourse.dve_uop import DveOpSpec

    if "EXP8_APPROX_ANT" in dve_ops._SUB_OPCODE_FOR_NAME:
        return next(op for op in dve_ops.OPS if op.name == "EXP8_APPROX_ANT")

    from concourse.dve_spec import Src0

    u = Src0 * C0
    body = sq(sq(sq((u * C1 + C2) * u + C2)))

    def _ref(in0, in1, s0, s1, imm2):
        uu = in0.astype(np.float32) * s0
        p = (uu * s1 + imm2) * uu + imm2
        return ((p * p) ** 2) ** 2

    spec = Spec(body=body, reference=_ref)

    row = max(dve_ops._SUB_OPCODE_FOR_NAME.values()) + 1
    assert row < 0x20

    # pin the sha by lowering once per version
    shas = {}
    for ver in ("v3", "v4"):
        try:
            uops = lower(spec, ver=ver)
            shas[ver] = DveOpSpec(
                name="EXP8_APPROX_ANT", opcode=row, uops=uops, rd1_en=False
            ).sha(ver)
        except Exception:
            pass

    op = DveOp("EXP8_APPROX_ANT", spec, subdim=False, uops_sha=shas)
    dve_ops.OPS.append(op)
    dve_ops.CUSTOM_DVE_SPECS[op.name] = spec
    dve_ops._SUB_OPCODE_FOR_NAME[op.name] = row
    return op


def _build():
    import concourse.bass as bass
    import concourse.mybir as mybir
    import concourse.tile as tile
    from concourse import bacc

    f32 = mybir.dt.float32
    bf16 = mybir.dt.bfloat16
    Exp = mybir.ActivationFunctionType.Exp
    ts = bass.ts

    exp8 = _register_exp8()
    exp8_s0 = SCALE / 8.0
    use_dve_exp = not bool(int(os.environ.get("ATTN_NO_DVE_EXP", "0")))

    nc = bacc.Bacc("TRN2", target_bir_lowering=False, debug=False,
                   num_devices=NCORES)

    xt = nc.dram_tensor("xt", [DIM, N], bf16, kind="ExternalInput")
    wqkv = nc.dram_tensor("wqkv", [DIM, 3 * DIM], bf16, kind="ExternalInput")
    wout = nc.dram_tensor("wout", [DIM, DIM], bf16, kind="ExternalInput")
    bout = nc.dram_tensor("bout", [128, 4], f32, kind="ExternalInput")
    yt = nc.dram_tensor("out", [DIM, NQ], f32, kind="ExternalOutput")

    with tile.TileContext(nc) as tc:
        with (
            tc.tile_pool(name="persist", bufs=1) as persist,
            tc.tile_pool(name="ptiles", bufs=8) as ptiles,
            tc.tile_pool(name="norm", bufs=2) as norm,
            tc.tile_pool(name="ysb", bufs=2) as ysb,
            tc.tile_pool(name="psum_s", bufs=1, space="PSUM") as psum_s,
            tc.tile_pool(name="psum_o", bufs=4, space="PSUM") as psum_o,
            tc.tile_pool(name="dram", bufs=2, space="DRAM") as dram,
        ):
            # ---- load inputs (fine-grained DMAs spread across queues) ----
            # Issue order = queue assignment order: land the chunks the first
            # projection (K tile 0, then Q, m=0) needs before everything else.
            xt_sb = persist.tile([128, 4, N], bf16)
            w_sb = persist.tile([128, 4, 3 * DIM], bf16)
            # critical path first, on distinct queues: the K m=0 / Q m=0
            # weight slivers and x^T (full rows; descriptor count is per
            # partition, so chunking columns does not speed a queue up)
            def ld(i, dst, srcs):
                eng = nc.sync if i % 2 == 0 else nc.scalar
                eng.dma_start(dst, srcs)
            for kt in range(4):
                ld(kt, w_sb[:, kt, 512:640], wqkv[ts(kt, 128), 512:640])
            for kt in range(4):
                ld(kt, xt_sb[:, kt, 0:1024], xt[ts(kt, 128), 0:1024])
            for kt in range(4):
                ld(kt + 1, w_sb[:, kt, 0:256], wqkv[ts(kt, 128), 0:256])
            for kt in range(4):
                ld(kt + 1, xt_sb[:, kt, 1024:2048], xt[ts(kt, 128), 1024:2048])
            for kt in range(4):
                ld(kt, w_sb[:, kt, 640:768], wqkv[ts(kt, 128), 640:768])
            for kt in range(4):
                ld(kt, w_sb[:, kt, ts(2, 512)], wqkv[ts(kt, 128), ts(2, 512)])
            for kt in range(4):
                ld(kt + 1, w_sb[:, kt, 768:1024], wqkv[ts(kt, 128), 768:1024])
            for kt in range(4):
                ld(kt, w_sb[:, kt, 256:512], wqkv[ts(kt, 128), 256:512])
            wout_sb = persist.tile([128, 4, DIM], bf16)
            for kt in range(4):
                ld(kt + 1, wout_sb[:, kt, :], wout[ts(kt, 128), :])
            bout_sb = persist.tile([128, 4], f32)
            nc.sync.dma_start(bout_sb[:], bout[:, :])

            qt_sb = persist.tile([128, 4, NQ], bf16)
            kt_sb = persist.tile([128, 4, N], bf16)
            # partition-swapped copies (head A on rows 64:128, B on rows
            # 0:64): chunk c0 of a head's score matmul runs on one PE row
            # half and chunk c1 on the other, so the two chunks pair on the
            # PE regardless of scheduler phase.
            qt_sw = persist.tile([128, 4, NQ], bf16)
            kt_sw = persist.tile([128, 4, N], bf16)

            def swap_kq(m, eng=None):
                # SBUF->SBUF partition-shift DMAs, chunked for early tiles.
                # At startup the sync/scalar queues are deep in input loads,
                # so the preamble swaps ride the shorter gpsimd queue.
                nc_sync = eng if eng is not None else nc.sync
                for c0 in range(0, N, 512):
                    nc_sync.dma_start(kt_sw[64:128, m, c0:c0 + 512],
                                      kt_sb[0:64, m, c0:c0 + 512])
                    nc_sync.dma_start(kt_sw[0:64, m, c0:c0 + 512],
                                      kt_sb[64:128, m, c0:c0 + 512])
                for c0 in range(0, NQ, 512):
                    nc_sync.dma_start(qt_sw[64:128, m, c0:c0 + 512],
                                      qt_sb[0:64, m, c0:c0 + 512])
                    nc_sync.dma_start(qt_sw[0:64, m, c0:c0 + 512],
                                      qt_sb[64:128, m, c0:c0 + 512])

            def se_copy(dst, src):
                nc.scalar.copy(dst, src)

            def ve_copy(dst, src):
                nc.vector.tensor_copy(dst, src)

            def q_proj_chunk(m, c, tag, copy_eng=se_copy):
                """One [128, 512] Q^T projection chunk: 4 accumulating
                matmuls + PSUM->SBUF copy.  tag picks the psum ring ('pv'
                during startup when the accumulators are idle, 'pj' for
                the single-slot in-loop ring)."""
                ps = psum_o.tile([128, 512], mybir.dt.float32, tag=tag,
                                 bufs=4, name=f"psq{m}c{c}")
                for kt in range(4):
                    nc.tensor.matmul(
                        ps[:, :],
                        lhsT=w_sb[:, kt, ts(m, 128)],
                        rhs=xt_sb[:, kt, ts(c, 512)],
                        start=(kt == 0), stop=(kt == 3),
                    )
                copy_eng(qt_sb[:, m, c * 512:(c + 1) * 512], ps[:, :])

            def k_proj_chunk(m, cc, c, tag, copy_eng=se_copy):
                base = cc * 1024 + c * 512
                ps = psum_o.tile([128, 512], mybir.dt.float32, tag=tag,
                                 bufs=4, name=f"psk{m}_{cc}c{c}")
                for kt in range(4):
                    nc.tensor.matmul(
                        ps[:, :],
                        lhsT=w_sb[:, kt, 512 + m * 128:512 + (m + 1) * 128],
                        rhs=xt_sb[:, kt, base:base + 512],
                        start=(kt == 0), stop=(kt == 3),
                    )
                copy_eng(kt_sb[:, m, base:base + 512], ps[:, :])

            # Preamble projections: pair 0/1 K and Q.  The pv ring (3
            # slots) is idle until pair 0's first PV, so most chunks cycle
            # through it; every third chunk uses the 1-slot pj ring.
            k_proj_chunk(0, 0, 0, "po", ve_copy)
            k_proj_chunk(0, 0, 1, "po", ve_copy)
            q_proj_chunk(0, 0, "po", ve_copy)
            q_proj_chunk(0, 1, "po", ve_copy)
            k_proj_chunk(0, 1, 0, "po")
            k_proj_chunk(0, 1, 1, "po")
            swap_kq(0)
            q_proj_chunk(1, 0, "po")
            q_proj_chunk(1, 1, "po")
            k_proj_chunk(1, 0, 0, "po")
            k_proj_chunk(1, 0, 1, "po")
            k_proj_chunk(1, 1, 0, "po")
            k_proj_chunk(1, 1, 1, "po")
            swap_kq(1)

            # V natural [2048, 512] -> v_sb [128, nk_tile, head, 64]; the
            # softmax denominators come from separate ones-matmuls (so the
            # per-head PV stationary stays 64-wide and two heads pair on
            # disjoint PE column groups).
            v_sb = persist.tile([128, 16, H, D], bf16)
            ones_sb = persist.tile([128, 1], bf16)
            nc.vector.memset(ones_sb[:, :], 1.0)

            def v_proj_tile(t):
                # just-in-time: emitted at iteration t of pair 0, consumed
                # by the PV rounds of iteration t+1
                ps = psum_o.tile([128, 512], mybir.dt.float32, tag="po",
                                 bufs=4, name=f"psv{t}")
                for kt in range(4):
                    nc.tensor.matmul(
                        ps[:, :],
                        lhsT=xt_sb[:, kt, ts(t, 128)],
                        rhs=w_sb[:, kt, 1024:1536],
                        start=(kt == 0), stop=(kt == 3),
                    )
                if t % 2 == 0:
                    nc.scalar.copy(
                        v_sb[:, t, :, :],
                        ps[:, :].rearrange("p (h d) -> p h d", h=H),
                    )
                else:
                    nc.vector.tensor_copy(
                        v_sb[:, t, :, :],
                        ps[:, :].rearrange("p (h d) -> p h d", h=H),
                    )

            # Deferred normalization phase-2: the reciprocal/broadcast/mul
            # for pair hp are emitted DURING pair hp+1's t-loop so the DVE
            # queue never stalls on the DMA round-trip (head-of-line
            # blocking at pair boundaries).
            pending_norm = []

            def norm_phase2a(pend):
                sp, hp_, dm0, dm1 = pend["sp"], pend["hp"], pend["dm0"], pend["dm1"]
                rsp = norm.tile([128, 16], mybir.dt.float32, tag="rsp", bufs=4)
                nc.vector.reciprocal(rsp[:, :], sp[:, :])
                sdA = dram.tile([1, NQ], mybir.dt.float32, tag="sdA", bufs=4)
                sdB = dram.tile([1, NQ], mybir.dt.float32, tag="sdB", bufs=4)
                dm0.dma_start(sdA[:, :], rsp[:, 0:8])
                dm1.dma_start(sdB[:, :], rsp[:, 8:16])
                bc = norm.tile([128, NQ], mybir.dt.float32, tag="bc", bufs=4)
                bcA = bass.AP(tensor=sdA.tensor, offset=sdA.offset,
                              ap=[[0, 64], [1, NQ]])
                bcB = bass.AP(tensor=sdB.tensor, offset=sdB.offset,
                              ap=[[0, 64], [1, NQ]])
                dm0.dma_start(bc[0:64, :], bcA)
                dm1.dma_start(bc[64:128, :], bcB)
                pend["bc"] = bc

            def norm_phase2b(pend):
                nc.vector.tensor_mul(ot_sb[:, pend["hp"], :],
                                     pend["oa"][:, :], pend["bc"][:, :])

            # ---- attention, one head PAIR at a time ----
            # Head A lives on partitions 0:64, head B on 64:128 of K^T/Q^T
            # tile hp.  Both heads' scores for one k-tile land in ONE
            # [128, 2048] PSUM tile (A cols 0:1024, B cols 1024:2048): the
            # 4 score matmuls become ready atomically, so the scheduler
            # issues them back-to-back and the PE row-tiles A/B pairs
            # concurrently.  exp(A) on ScalarE reads banks 0-1 while
            # exp8(B) on VectorE reads banks 2-3 in parallel.
            ot_sb = persist.tile([128, 4, NQ], bf16)

            # PV accumulators for the pair currently being accumulated:
            # pv0 = c0 bank (A rows 0:64, B rows 64:128), pv1 = c1 bank,
            # pden = denominator bank with rows {0: A-c0, 32: B-c0,
            # 64: A-c1, 96: B-c1}.
            cur_pv = {}

            def pv_round(prev, rnd):
                pA, pB, pt_, php = prev
                if not cur_pv or cur_pv["hp"] != php:
                    cur_pv.clear()
                    cur_pv.update(
                        hp=php,
                        pv0=psum_o.tile([128, 512], mybir.dt.float32,
                                        tag="po", bufs=4, name="pv0"),
                        pv1=psum_o.tile([128, 512], mybir.dt.float32,
                                        tag="po", bufs=4, name="pv1"),
                        pden=psum_o.tile([128, 512], mybir.dt.float32,
                                         tag="po", bufs=4, name="pden"),
                    )
                pv0, pv1, pden = cur_pv["pv0"], cur_pv["pv1"], cur_pv["pden"]
                hA_, hB_ = 2 * php, 2 * php + 1
                st, sp_ = (pt_ == 0), (pt_ == 15)
                if rnd == 0:
                    nc.tensor.matmul(
                        pv0[0:64, :], lhsT=v_sb[:, pt_, hA_, :],
                        rhs=pA[:, 0:512], start=st, stop=sp_,
                        tile_position=(0, 0),
                    )
                    nc.tensor.matmul(
                        pv0[64:128, :], lhsT=v_sb[:, pt_, hB_, :],
                        rhs=pB[:, 0:512], start=st, stop=sp_,
                        tile_position=(0, 64),
                    )
                elif rnd == 1:
                    nc.tensor.matmul(
                        pv1[0:64, :], lhsT=v_sb[:, pt_, hA_, :],
                        rhs=pA[:, 512:1024], start=st, stop=sp_,
                        tile_position=(0, 0),
                    )
                    nc.tensor.matmul(
                        pv1[64:128, :], lhsT=v_sb[:, pt_, hB_, :],
                        rhs=pB[:, 512:1024], start=st, stop=sp_,
                        tile_position=(0, 64),
                    )
                else:
                    nc.tensor.matmul(
                        pden[0:1, :], lhsT=ones_sb[:, :],
                        rhs=pA[:, 0:512], start=st, stop=sp_,
                        tile_position=(0, 0),
                    )
                    nc.tensor.matmul(
                        pden[32:33, :], lhsT=ones_sb[:, :],
                        rhs=pB[:, 0:512], start=st, stop=sp_,
                        tile_position=(0, 32),
                    )
                    nc.tensor.matmul(
                        pden[64:65, :], lhsT=ones_sb[:, :],
                        rhs=pA[:, 512:1024], start=st, stop=sp_,
                        tile_position=(0, 64),
                    )
                    nc.tensor.matmul(
                        pden[96:97, :], lhsT=ones_sb[:, :],
                        rhs=pB[:, 512:1024], start=st, stop=sp_,
                        tile_position=(0, 96),
                    )

            def norm_phase1(php):
                # evacuate PSUM: denominators first, then the two O banks;
                # spread the sums across partitions for a wide reciprocal.
                pv0, pv1, pden = cur_pv["pv0"], cur_pv["pv1"], cur_pv["pden"]
                # pair 3's chain is the endgame critical path: keep it on
                # the two hardware-DGE queues (gpsimd DMAs are SWDGE-slow)
                dm0 = nc.scalar if php == 3 else nc.sync
                dm1 = nc.sync if php == 3 else nc.gpsimd
                den_sb = norm.tile([128, 512], mybir.dt.float32, tag="den")
                nc.scalar.copy(den_sb[:, :], pden[:, :])
                oa = norm.tile([128, NQ], mybir.dt.float32, tag="oa")
                nc.scalar.copy(oa[:, 0:512], pv0[:, :])
                nc.vector.tensor_copy(oa[:, 512:NQ], pv1[:, :])
                sp = norm.tile([128, 16], mybir.dt.float32, tag="sp", bufs=4)
                dm0.dma_start(sp[0:64, 0:8], den_sb[0:1, :])
                dm0.dma_start(sp[64:128, 0:8], den_sb[64:65, :])
                dm1.dma_start(sp[0:64, 8:16], den_sb[32:33, :])
                dm1.dma_start(sp[64:128, 8:16], den_sb[96:97, :])
                return {"sp": sp, "oa": oa, "hp": php, "dm0": dm0, "dm1": dm1}

            # ---- attention, one head pair at a time ----
            # Per iteration the PE FIFO order is [scores-c0 pair, PV-I1,
            # scores-c1 pair, PV-I2, PV-dens]: each group's inputs were
            # released exactly one exp-chunk earlier, so the PE never
            # stalls on a not-yet-finished exp while ready work waits
            # behind it.
            for hp in range(4):
                prev = None
                for t in range(17):
                    if t == 4 and pending_norm:
                        norm_phase2a(pending_norm[-1])
                    if t == 10 and pending_norm:
                        norm_phase2b(pending_norm.pop())
                    if t < 16:
                        sA0 = psum_s.tile([128, 512], mybir.dt.float32,
                                          tag="sA0", name="sA0")
                        sA1 = psum_s.tile([128, 512], mybir.dt.float32,
                                          tag="sA1", name="sA1")
                        sB0 = psum_s.tile([128, 512], mybir.dt.float32,
                                          tag="sB0", name="sB0")
                        sB1 = psum_s.tile([128, 512], mybir.dt.float32,
                                          tag="sB1", name="sB1")
                        nc.tensor.matmul(
                            sA0[:, :],
                            lhsT=kt_sb[0:64, hp, ts(t, 128)],
                            rhs=qt_sb[0:64, hp, 0:512],
                            start=True, stop=True,
                        )
                        nc.tensor.matmul(
                            sB0[:, :],
                            lhsT=kt_sb[64:128, hp, ts(t, 128)],
                            rhs=qt_sb[64:128, hp, 0:512],
                            start=True, stop=True,
                        )
                    if t < 16:
                        nc.tensor.matmul(
                            sA1[:, :],
                            lhsT=kt_sw[64:128, hp, ts(t, 128)],
                            rhs=qt_sw[64:128, hp, 512:1024],
                            start=True, stop=True,
                        )
                        nc.tensor.matmul(
                            sB1[:, :],
                            lhsT=kt_sw[0:64, hp, ts(t, 128)],
                            rhs=qt_sw[0:64, hp, 512:1024],
                            start=True, stop=True,
                        )
                    if t < 16:
                        ptA = ptiles.tile([128, NQ], bf16, tag="pt")
                        ptB = ptiles.tile([128, NQ], bf16, tag="pt")
                        nc.scalar.activation(ptA[:, 0:512], sA0[:, :], Exp,
                                             scale=SCALE)
                        if use_dve_exp:
                            nc.vector._custom_dve(
                                exp8, out=ptB[:, 0:512], in0=sB0[:, :],
                                s0=exp8_s0, s1=EXP8_C1, imm2=1.0,
                            )
                        else:
                            nc.scalar.activation(ptB[:, 0:512], sB0[:, :],
                                                 Exp, scale=SCALE)
                        nc.scalar.activation(ptA[:, 512:NQ], sA1[:, :], Exp,
                                             scale=SCALE)
                        if use_dve_exp:
                            nc.vector._custom_dve(
                                exp8, out=ptB[:, 512:NQ], in0=sB1[:, :],
                                s0=exp8_s0, s1=EXP8_C1, imm2=1.0,
                            )
                        else:
                            nc.scalar.activation(ptB[:, 512:NQ], sB1[:, :],
                                                 Exp, scale=SCALE)
                    if prev is not None:
                        pv_round(prev, 0)
                        pv_round(prev, 1)
                        pv_round(prev, 2)
                    # V-projection rides here for pair 0: the first score
                    # group + exp are already emitted, so ScalarE ramps up
                    # while the PE grinds through the V matmuls.
                    if hp == 0 and t == 0:
                        for vt in range(16):
                            v_proj_tile(vt)
                    if t < 16:
                        prev = (ptA, ptB, t, hp)
                pend = norm_phase1(hp)
                if hp == 3:
                    # last pair: no next t-loop to hide behind; run the
                    # chain immediately, split per 512-chunk so the c0
                    # multiply (and the tail's final contraction for c0)
                    # starts as soon as the first broadcast lands.
                    sp3, oa3 = pend["sp"], pend["oa"]
                    rsp = norm.tile([128, 16], mybir.dt.float32, tag="rsp",
                                    bufs=4)
                    nc.vector.reciprocal(rsp[:, :], sp3[:, :])
                    sdA = dram.tile([1, NQ], mybir.dt.float32, tag="sdA",
                                    bufs=4)
                    sdB = dram.tile([1, NQ], mybir.dt.float32, tag="sdB",
                                    bufs=4)
                    bc3 = norm.tile([128, NQ], mybir.dt.float32, tag="bc",
                                    bufs=4)
                    for cc, (p0, p1) in enumerate(((0, 64), (64, 128))):
                        nc.scalar.dma_start(sdA[:, cc * 512:(cc + 1) * 512],
                                            rsp[p0:p1, 0:8])
                        nc.sync.dma_start(sdB[:, cc * 512:(cc + 1) * 512],
                                            rsp[p0:p1, 8:16])
                        bcA = bass.AP(tensor=sdA.tensor,
                                      offset=sdA.offset + cc * 512,
                                      ap=[[0, 64], [1, 512]])
                        bcB = bass.AP(tensor=sdB.tensor,
                                      offset=sdB.offset + cc * 512,
                                      ap=[[0, 64], [1, 512]])
                        cs3 = ts(cc, 512)
                        nc.scalar.dma_start(bc3[0:64, cs3], bcA)
                        nc.sync.dma_start(bc3[64:128, cs3], bcB)
                        nc.vector.tensor_mul(ot_sb[:, 3, cs3],
                                             oa3[:, cs3], bc3[:, cs3])
                else:
                    pending_norm.append(pend)
                # next pair's projections ride the pair boundary: the po
                # ring slots are freeing up (den/oa copies done)
                if hp < 2:
                    nx = hp + 2
                    q_proj_chunk(nx, 0, "po")
                    q_proj_chunk(nx, 1, "po", ve_copy)
                    k_proj_chunk(nx, 0, 0, "po")
                    k_proj_chunk(nx, 0, 1, "po", ve_copy)
                    k_proj_chunk(nx, 1, 0, "po")
                    k_proj_chunk(nx, 1, 1, "po", ve_copy)
                    swap_kq(nx)
            # ---- tail: full output projection Y^T = Wout^T @ O^T.
            # Pairs 0..2 are accumulated for ALL e-tiles as soon as the last
            # pair's score/exp traffic frees the PSUM slots -- this runs
            # UNDER pair 3's normalization DMA round-trips.  Only the pair-3
            # contraction, bias and store wait for the final norm.
            tail_ps = []
            for m in range(4):
                if m == 0:
                    c0 = psum_s.tile([128, 512], mybir.dt.float32, tag="sA0",
                                     name=f"psyT{m}c0")
                    c1 = psum_s.tile([128, 512], mybir.dt.float32, tag="sA1",
                                     name=f"psyT{m}c1")
                    chunks = [c0[:, :], c1[:, :]]
                elif m == 1:
                    c0 = psum_s.tile([128, 512], mybir.dt.float32, tag="sB0",
                                     name=f"psyT{m}c0")
                    c1 = psum_s.tile([128, 512], mybir.dt.float32, tag="sB1",
                                     name=f"psyT{m}c1")
                    chunks = [c0[:, :], c1[:, :]]
                else:
                    c0 = psum_o.tile([128, 512], mybir.dt.float32, tag="po",
                                     bufs=4, name=f"psyT{m}c0")
                    c1 = psum_o.tile([128, 512], mybir.dt.float32, tag="po",
                                     bufs=4, name=f"psyT{m}c1")
                    chunks = [c0[:, :], c1[:, :]]
                for c in range(2):
                    cs = ts(c, 512)
                    for hp in range(3):
                        nc.tensor.matmul(
                            chunks[c],
                            lhsT=wout_sb[:, hp, ts(m, 128)],
                            rhs=ot_sb[:, hp, cs],
                            start=(hp == 0), stop=False,
                        )
                tail_ps.append(chunks)
            # c-major: the whole c0 half (final contraction, bias, store)
            # completes while c1's normalization broadcast is still landing
            ys_t = [ysb.tile([128, NQ], mybir.dt.float32, tag="ys", bufs=4,
                             name=f"ys{m}") for m in range(4)]
            for c in range(2):
                cs = ts(c, 512)
                for m in range(4):
                    nc.tensor.matmul(
                        tail_ps[m][c],
                        lhsT=wout_sb[:, 3, ts(m, 128)],
                        rhs=ot_sb[:, 3, cs],
                        start=False, stop=True,
                    )
                    nc.vector.tensor_scalar_add(ys_t[m][:, cs],
                                                tail_ps[m][c],
                                                bout_sb[:, m:m + 1])
                    nc.sync.dma_start(yt[ts(m, 128), cs], ys_t[m][:, cs])

    nc.compile()
    return nc


def _get_nc():
    if "nc" not in _CACHE:
        _CACHE["nc"] = _build()
    return _CACHE["nc"]


def kernel(x, w_qkv, w_out, b_out):
    global LAST_EXEC_TIME_NS
    from concourse.bass_utils import run_bass_kernel_spmd

    x = np.asarray(x, dtype=np.float32)
    w_qkv = np.asarray(w_qkv, dtype=np.float32)
    w_out = np.asarray(w_out, dtype=np.float32)
    b_out = np.asarray(b_out, dtype=np.float32)

    wqkv_b = w_qkv.astype(BF16)
    wout_b = w_out.astype(BF16)
    bout_t = np.ascontiguousarray(b_out.reshape(4, 128).T).astype(np.float32)

    in_maps = []
    for c in range(NCORES):
        b, qh = c // 2, c % 2
        q0 = qh * NQ
        xb = x[b]
        perm = np.concatenate([
            np.arange(q0, q0 + NQ),
            np.arange(0, q0),
            np.arange(q0 + NQ, N),
        ])
        xt = np.ascontiguousarray(xb[perm].T).astype(BF16)
        in_maps.append({
            "xt": xt,
            "wqkv": wqkv_b,
            "wout": wout_b,
            "bout": bout_t,
        })

    nc = _get_nc()
    trace = bool(int(os.environ.get("ATTN_TRACE", "0")))
    res = run_bass_kernel_spmd(nc, in_maps, core_ids=list(range(NCORES)),
                               trace=trace)
    LAST_EXEC_TIME_NS = res.exec_time_ns

    out = np.empty((B, N, DIM), np.float32)
    for c in range(NCORES):
        b, qh = c // 2, c % 2
        out[b, qh * NQ:(qh + 1) * NQ, :] = res.results[c]["out"].T
    return out

